# revision 6
# baseline (speedup 1.0000x reference)
"""Trainium2 Bass kernel for nn_Conv2dMem: bit-plane quantized 3x3 conv.

Math: the reference quantizes input and weight to 8-bit fixed point
(scale = absmax/127), splits each into two 4-bit planes, and sums the four
plane-pair GEMMs with power-of-two recombination. By bilinearity that sum
equals the single GEMM of the full quantized operands, and every
intermediate is an exact integer below 2^24 (|q| <= 127, K = 576), so one
bf16 x bf16 -> fp32 PE matmul reproduces the reference bit-for-bit.

Layout per core (8 cores, 2 images each):
  - full input [128, 8192] fp32 in HBM, rolled so this core's shard is
    columns 0:1024; partition = b_local*64 + channel. The whole tensor is
    reduced for the global abs-max (the reference's scale is global), only
    the shard is quantized.
  - conv as shift-conv: zero-padded quantized image planes (34x34) on
    partitions 0..63, the same planes shifted by one padded row on
    partitions 64..127. A K=128 matmul then contracts taps (0,kw)+(1,kw)
    over 64 channels in one shot; taps (2,kw) use K=64. 6 matmuls per
    [128cout, 512pos] PSUM tile, 4 tiles (2 images x 2 row-blocks).
"""

import numpy as np

import concourse.bass as bass
import concourse.mybir as mybir
import concourse.tile as tile
from concourse.bass_utils import run_bass_kernel_spmd

N_CORES = 8
B, CIN, COUT, H, W = 16, 64, 128, 32, 32
B_LOC = B // N_CORES          # 2 images per core
HP, WP = H + 2, W + 2         # 34x34 padded plane
PLANE = HP * WP               # 1156
L = H * W                     # 1024 output positions per image
MAGIC = float(np.float32(12582912.0))      # 1.5 * 2**23, fp32 RNE rounder
C127 = float(np.float32(1.0) / np.float32(127.0))

_NC_CACHE = None
TRACE = False
LAST_EXEC_NS = None
LAST_RESULT = None


def _fix_multiwaits(nc, max_waits=1):
    """This container's walrus accepts at most one semaphore wait per
    instruction; TileContext emits more on the tail drain / DMA triggers.
    Hoist extras onto fresh same-engine NOPs placed just before (engines
    are in-order, so this is semantically identical)."""
    n = 0
    for fn in nc.m.functions:
        for bb in fn.blocks:
            insts = bb.instructions
            if not any(
                i.sync_info is not None and len(i.sync_info.on_wait) > max_waits
                for i in insts
            ):
                continue
            new_list = []
            for ins in insts:
                si = ins.sync_info
                if si is not None and len(si.on_wait) > max_waits:
                    waits = list(si.on_wait)
                    extra, keep = waits[:-max_waits], waits[-max_waits:]
                    for i, w in enumerate(extra):
                        new_list.append(
                            mybir.InstNoOp(
                                name=f"{ins.name}-hw{i}",
                                engine=ins.engine,
                                sync_info=mybir.SyncInfo(on_wait=[w], on_update=[]),
                                bass_nofuse=True,
                            )
                        )
                    ins.sync_info = mybir.SyncInfo(
                        on_wait=keep, on_update=list(si.on_update)
                    )
                    n += 1
                new_list.append(ins)
            bb.instructions = new_list
    return n


def _build_nc(fix_waits=True):
    f32, bf16 = mybir.dt.float32, mybir.dt.bfloat16
    nc = bass.Bass(target_bir_lowering=True)

    xfull = nc.dram_tensor("xfull", [128, 8 * 1024], f32, kind="ExternalInput")
    wt = nc.dram_tensor("wt", [64, 9 * 128], f32, kind="ExternalInput")
    bias = nc.dram_tensor("bias", [128, 1], f32, kind="ExternalInput")
    out = nc.dram_tensor("out", [B_LOC, COUT, L], f32, kind="ExternalOutput")

    with tile.TileContext(nc) as tc:
        with (
            tc.tile_pool(name="pool", bufs=1) as pool,
            tc.tile_pool(name="psum", bufs=1, space="PSUM") as psum_pool,
        ):
            # ---- loads -------------------------------------------------
            bias_t = pool.tile([128, 1], f32, tag="bias")
            nc.sync.dma_start(bias_t[:], bias[:])

            wt_t = pool.tile([64, 9 * 128], f32, tag="wt")
            nc.sync.dma_start(wt_t[:], wt[:])

            xf = pool.tile([128, 8 * 1024], f32, tag="xf")
            pm = pool.tile([128, 8], f32, tag="pm")
            for j in range(8):
                sl = slice(j * 1024, (j + 1) * 1024)
                nc.sync.dma_start(xf[:, sl], xfull[:, sl])
                nc.vector.tensor_reduce(
                    pm[:, j : j + 1], xf[:, sl], axis=mybir.AxisListType.X,
                    op=mybir.AluOpType.max, apply_absolute_value=True,
                )

            # ---- global abs-max scalars --------------------------------
            xm = pool.tile([128, 1], f32, tag="xm")
            nc.vector.tensor_reduce(
                xm[:], pm[:], axis=mybir.AxisListType.X,
                op=mybir.AluOpType.max,
            )
            wm = pool.tile([64, 1], f32, tag="wm")
            nc.vector.tensor_reduce(
                wm[:], wt_t[:], axis=mybir.AxisListType.X,
                op=mybir.AluOpType.max, apply_absolute_value=True,
            )
            gx = pool.tile([1, 128], f32, tag="gx")
            nc.sync.dma_start(gx[:], xm[:, 0])
            gw = pool.tile([1, 64], f32, tag="gw")
            nc.sync.dma_start(gw[:], wm[:, 0])

            sc = pool.tile([1, 8], f32, tag="sc")   # Mx, Mw, invMx, invMw, sx, sw
            nc.vector.tensor_reduce(
                sc[0:1, 0:1], gx[0:1, :], axis=mybir.AxisListType.X,
                op=mybir.AluOpType.max,
            )
            nc.vector.tensor_reduce(
                sc[0:1, 1:2], gw[0:1, :], axis=mybir.AxisListType.X,
                op=mybir.AluOpType.max,
            )
            nc.vector.reciprocal(sc[0:1, 2:3], sc[0:1, 0:1])
            nc.vector.reciprocal(sc[0:1, 3:4], sc[0:1, 1:2])
            nc.vector.tensor_scalar(
                sc[0:1, 4:5], sc[0:1, 0:1], C127, None, op0=mybir.AluOpType.mult
            )  # s_x = fl(Mx * fl(1/127)) == fl(Mx/127)
            nc.vector.tensor_scalar(
                sc[0:1, 5:6], sc[0:1, 1:2], C127, None, op0=mybir.AluOpType.mult
            )  # s_w

            bs = pool.tile([1, 3], f32, tag="bs")   # r_x, r_w, p
            nc.vector.tensor_scalar(
                bs[0:1, 0:1], sc[0:1, 2:3], 127.0, None, op0=mybir.AluOpType.mult
            )  # r_x = fl(127 * fl(1/Mx)) == fl(1/s_x)
            nc.vector.tensor_scalar(
                bs[0:1, 1:2], sc[0:1, 3:4], 127.0, None, op0=mybir.AluOpType.mult
            )  # r_w
            nc.vector.tensor_mul(bs[0:1, 2:3], sc[0:1, 4:5], sc[0:1, 5:6])

            ones = pool.tile([1, 128], f32, tag="ones")
            nc.vector.memset(ones[:1, :], 1.0)
            ps_b = psum_pool.tile([128, 3], f32, tag="psb")
            nc.tensor.matmul(ps_b[:], ones[0:1, :], bs[0:1, 0:3], start=True, stop=True)
            bc = pool.tile([128, 3], f32, tag="bc")   # broadcast r_x, r_w, p
            nc.vector.tensor_copy(bc[:], ps_b[:])

            # ---- quantize weight (64 partitions: channel-major) --------
            wq1 = pool.tile([64, 9 * 128], f32, tag="wq1")
            nc.vector.tensor_scalar(
                wq1[:], wt_t[:], bc[0:64, 1:2], MAGIC,
                op0=mybir.AluOpType.mult, op1=mybir.AluOpType.add,
            )
            wq2 = pool.tile([64, 9 * 128], f32, tag="wq2")
            nc.vector.tensor_scalar(
                wq2[:], wq1[:], -MAGIC, -127.0,
                op0=mybir.AluOpType.add, op1=mybir.AluOpType.max,
            )
            wq = pool.tile([64, 9 * 128], bf16, tag="wq")
            nc.vector.tensor_scalar(
                wq[:], wq2[:], 127.0, None, op0=mybir.AluOpType.min
            )

            # lhsT tiles: pairs [128, 3*128] = taps (0,kw)+(1,kw); singles
            # [64, 3*128] = taps (2,kw). wq free index = (kh*3+kw)*128 + co,
            # so each kh is one contiguous 384-col slice.
            wl = pool.tile([128, 2 * 3 * 128], bf16, tag="wl")
            for kh in range(2):
                nc.sync.dma_start(
                    wl[kh * 64 : (kh + 1) * 64, 0:384],
                    wq[:, kh * 384 : (kh + 1) * 384],
                )
            nc.sync.dma_start(wl[0:64, 384:768], wq[:, 768:1152])

            # ---- quantize input shard (xf cols 0:1024) ------------------
            xq1 = pool.tile([128, 1024], f32, tag="xq1")
            nc.vector.tensor_scalar(
                xq1[:], xf[:, 0:1024], bc[:, 0:1], MAGIC,
                op0=mybir.AluOpType.mult, op1=mybir.AluOpType.add,
            )
            xq2 = pool.tile([128, 1024], f32, tag="xq2")
            nc.vector.tensor_scalar(
                xq2[:], xq1[:], -MAGIC, -127.0,
                op0=mybir.AluOpType.add, op1=mybir.AluOpType.max,
            )
            qxd = pool.tile([128, 1024], bf16, tag="qxd")
            nc.vector.tensor_scalar(
                qxd[:], xq2[:], 127.0, None, op0=mybir.AluOpType.min
            )

            # ---- padded planes + row-shifted copy ----------------------
            qpad = pool.tile([128, B_LOC * PLANE], bf16, tag="qpad")
            nc.vector.memset(qpad[:], 0.0)
            # interior scatter: qpad[c, b*1156 + (h+1)*34 + (w+1)] = qxd[b*64+c, h*32+w]
            for b in range(B_LOC):
                qpad_int = qpad[0:64, b * PLANE : (b + 1) * PLANE].rearrange(
                    "c (r cc) -> c r cc", cc=WP
                )[:, 1 : H + 1, 1 : W + 1]
                qxd_v = qxd[b * 64 : (b + 1) * 64, :].rearrange(
                    "c (h w) -> c h w", w=W
                )
                nc.sync.dma_start(qpad_int, qxd_v)
            # bottom half = top shifted by one padded row (34 elements)
            top_v = qpad[0:64, :].rearrange("c (b r) -> c b r", b=B_LOC)
            bot_v = qpad[64:128, :].rearrange("c (b r) -> c b r", b=B_LOC)
            nc.sync.dma_start(
                bot_v[:, :, 0 : PLANE - WP], top_v[:, :, WP:PLANE]
            )

            # ---- conv matmuls ------------------------------------------
            qp_v = qpad[:].rearrange("p (b r cc) -> p b r cc", b=B_LOC, cc=WP)
            for b in range(B_LOC):
                for ohb in range(2):
                    ps = psum_pool.tile([128, 512], f32, tag="ps")
                    r0 = ohb * 16
                    for kw in range(3):
                        rhs = qp_v[:, b, r0 : r0 + 16, kw : kw + 32]
                        nc.tensor.matmul(
                            ps[:],
                            wl[:, kw * 128 : (kw + 1) * 128],
                            rhs,
                            start=(kw == 0),
                            stop=False,
                        )
                    for kw in range(3):
                        rhs = qp_v[0:64, b, r0 + 2 : r0 + 18, kw : kw + 32]
                        nc.tensor.matmul(
                            ps[:],
                            wl[0:64, (3 + kw) * 128 : (4 + kw) * 128],
                            rhs,
                            start=False,
                            stop=(kw == 2),
                        )
                    osb = pool.tile([128, 512], f32, tag="osb")
                    nc.vector.tensor_scalar(
                        osb[:], ps[:], bc[:, 2:3], bias_t[:, 0:1],
                        op0=mybir.AluOpType.mult, op1=mybir.AluOpType.add,
                    )
                    nc.sync.dma_start(
                        out[b, :, ohb * 512 : (ohb + 1) * 512], osb[:]
                    )

    if fix_waits:
        _fix_multiwaits(nc)
    return nc


def kernel(input, weight, bias):
    global _NC_CACHE
    if _NC_CACHE is None:
        _NC_CACHE = _build_nc()
    nc = _NC_CACHE

    x = np.ascontiguousarray(input, dtype=np.float32)
    w = np.ascontiguousarray(weight, dtype=np.float32)
    b = np.ascontiguousarray(bias, dtype=np.float32)

    # [8 cores, 2 b_loc, 64 c, 1024 px] -> per-core [128, 8192] with own
    # shard rolled to the front
    base = x.reshape(N_CORES, B_LOC, CIN, L).transpose(1, 2, 0, 3)  # [2,64,8,1024]
    wt_arr = np.ascontiguousarray(
        w.reshape(COUT, CIN, 9).transpose(1, 2, 0).reshape(CIN, 9 * COUT)
    )
    bias_arr = np.ascontiguousarray(b.reshape(COUT, 1))

    in_maps = []
    for i in range(N_CORES):
        rolled = np.ascontiguousarray(
            np.roll(base, -i, axis=2).reshape(128, N_CORES * L)
        )
        in_maps.append({"xfull": rolled, "wt": wt_arr, "bias": bias_arr})

    global LAST_EXEC_NS, LAST_RESULT
    res = run_bass_kernel_spmd(
        nc, in_maps, core_ids=list(range(N_CORES)), trace=TRACE
    )
    LAST_EXEC_NS = res.exec_time_ns
    LAST_RESULT = res
    out = np.empty((B, COUT, H, W), dtype=np.float32)
    for i in range(N_CORES):
        out[i * B_LOC : (i + 1) * B_LOC] = res.results[i]["out"].reshape(
            B_LOC, COUT, H, W
        )
    return out
